# revision 1
# baseline (speedup 1.0000x reference)
"""Trainium2 Bass kernel for circular 3x3 conv — host-packed bf16 GEMM form.

out[b,h,w,f] = sum_{dh,dw,c} x[b,(h-dh)%H,(w-dw)%W,c] * K[j*C+c, f] + bias[f]
with j = dhi + 3*dwi, dh = dhi-1, dw = dwi-1.

Strategy (cost-model-driven): every matmul costs out_free_size cycles on the
PE regardless of contraction depth, so pack the 9*C=576-deep contraction into
as few, widest matmuls as possible. Host pre-packs x into three bf16 layouts
so that each output column PAIR (w=2i, 2i+1) accumulates in one PSUM tile
[128h, 512=(2 cols x 256f)] with exactly 6 matmuls (4x N=512 + 2x N=256):

  MM1-3 (d=dhi): lhsT = xe[i] slab (cols 2i,2i+1 on partitions, padded h
         free) sliced at h-offset 2-d; rhs = T_d [128,512] covering
         (dw=0,-1) for col w and (dw=+1,0) for col w+1 — all 4 quadrants
         of the kernel tile are live.
  MM4:   lhsT = xq[2i-1]  (col w-1 at dh=-1,0 stacked); rhs = [K_6;K_7]
         -> po[:,0:256]   (col w's dw=+1 taps, dhi=0,1)
  MM5:   lhsT = xq[2i+2]  (col w+2);  rhs = [K_0;K_1] -> po[:,256:512]
  MM6:   lhsT = xr[i] = [col 2i-1 @ dh=+1 ; col 2i+2 @ dh=+1];
         rhs = [[K_8,0],[0,K_2]] (zero-masked) -> po[:,0:512]

DVE adds bias and casts to bf16; bulk 8-column DMAs write out. Host casts
the bf16 result back to fp32. Inputs/kernel in bf16 (rel err ~4e-4 << 2e-2).
"""
import numpy as np

B, H, W, C, F = 16, 128, 128, 64, 256
NCORES = 8
BPC = B // NCORES   # batches per core
NP = W // 2         # column pairs per batch
NCHUNK = 8          # input streaming chunks per batch


def _build_module():
    import concourse.bacc as bacc
    import concourse.mybir as mybir
    import concourse.tile as tile

    f32 = mybir.dt.float32
    bf16 = mybir.dt.bfloat16

    nc = bacc.Bacc("TRN2", target_bir_lowering=False, debug=False,
                   num_devices=NCORES)
    xe_d = nc.dram_tensor("xe", [BPC, 128, NP, 130], bf16,
                          kind="ExternalInput").ap()
    xq_d = nc.dram_tensor("xq", [BPC, 128, W, 128], bf16,
                          kind="ExternalInput").ap()
    xr_d = nc.dram_tensor("xr", [BPC, 128, NP, 128], bf16,
                          kind="ExternalInput").ap()
    # x0a carries the kernel tiles + exactly pair 1's slabs (first DMA, so
    # compute starts as early as possible); x0b the rest of chunk 0.
    # x0a: [6x512 kt | E1 130 | Q1 128 | Q4 128 | R1 128] = 3586
    # x0b: [E0 E2 E3 (3x130) | Q0 Q2 Q3 Q5 Q6 Q7 (6x128) | R0 R2 R3 (3x128)]
    x0a_d = nc.dram_tensor("x0a", [BPC, 128, 3586], bf16,
                          kind="ExternalInput").ap()
    x0b_d = nc.dram_tensor("x0b", [BPC, 128, 1542], bf16,
                          kind="ExternalInput").ap()
    biasf_d = nc.dram_tensor("biasf", [128, 512], f32,
                             kind="ExternalInput").ap()
    out_d = nc.dram_tensor("out", [BPC, H, W, F], bf16,
                           kind="ExternalOutput").ap()

    # Geometric chunk boundaries (slab index space): chunk 0 lives in x0;
    # later chunks stream from xe/xq/xr. xq is in column space (2x).
    ECH = [4, 16, 40, 64]
    QCH = [8, 32, 80, 128]
    WARMUP = 160

    with tile.TileContext(nc) as tc:
        with (
            tc.tile_pool(name="persist", bufs=1) as persist,
            tc.tile_pool(name="xdbl", bufs=2) as xdbl,
            tc.tile_pool(name="outp", bufs=4) as outp,
            tc.tile_pool(name="ps", bufs=7, space="PSUM") as ps,
            tc.tile_pool(name="psw", bufs=1, space="PSUM") as psw,
        ):
            NCH = len(ECH) - 1
            xe_sb = [[None] * NCH for _ in range(BPC)]
            xq_sb = [[None] * NCH for _ in range(BPC)]
            xr_sb = [[None] * NCH for _ in range(BPC)]

            def load_chunk(b, g):
                pool = xdbl if g < 1 else persist
                e0, e1 = ECH[g], ECH[g + 1]
                q0, q1 = QCH[g], QCH[g + 1]
                te = pool.tile([128, e1 - e0, 130], bf16, tag=f"xe_{g}")
                nc.sync.dma_start(te[:], xe_d[b, :, e0:e1, :])
                tq = pool.tile([128, q1 - q0, 128], bf16, tag=f"xq_{g}")
                nc.sync.dma_start(tq[:], xq_d[b, :, q0:q1, :])
                tr = pool.tile([128, e1 - e0, 128], bf16, tag=f"xr_{g}")
                nc.sync.dma_start(tr[:], xr_d[b, :, e0:e1, :])
                xe_sb[b][g] = te
                xq_sb[b][g] = tq
                xr_sb[b][g] = tr

            # Input DMA issue order: batch-0 first pairs ASAP, then batch-0
            # bulk, batch-1 interleaved behind. Early chunks double-buffered
            # so batch-1's loads run ahead; big chunks (2,3) single-buffered
            # so their batch-1 DMAs self-throttle on batch-0's readers
            # (keeps the DMA device free for output writes mid-flight).
            if WARMUP:
                # p-state warmup: tiny dep-free matmuls (source is a
                # memset scratch, result goes to a never-read PSUM bank)
                # start right after the entry barrier and keep the PE
                # continuously busy through the cost model's ramp window,
                # ending just as pair 1's data lands.
                wsrc = persist.tile([128, 64], bf16, tag="wsrc")
                nc.vector.memset(wsrc[:], 0.0)
                warm = psw.tile([32, 32], f32, tag="warm")
                for _ in range(WARMUP):
                    nc.tensor.matmul(warm[:], wsrc[:, 0:32], wsrc[:, 32:64],
                                     start=True, stop=True)

            x0a_sb, x0b_sb = [None] * BPC, [None] * BPC
            ta = xdbl.tile([128, 3586], bf16, tag="x0a")
            nc.sync.dma_start(ta[:], x0a_d[0, :, :])
            x0a_sb[0] = ta
            tb = xdbl.tile([128, 1542], bf16, tag="x0b")
            nc.sync.dma_start(tb[:], x0b_d[0, :, :])
            x0b_sb[0] = tb
            biasf = persist.tile([128, 512], f32, tag="biasf")
            nc.sync.dma_start(biasf[:], biasf_d[:])
            load_chunk(0, 0)
            load_chunk(0, 1)
            load_chunk(0, 2)
            ta = xdbl.tile([128, 3586], bf16, tag="x0a")
            nc.sync.dma_start(ta[:], x0a_d[1, :, :])
            x0a_sb[1] = ta
            tb = xdbl.tile([128, 1542], bf16, tag="x0b")
            nc.sync.dma_start(tb[:], x0b_d[1, :, :])
            x0b_sb[1] = tb
            load_chunk(1, 0)
            load_chunk(1, 1)
            load_chunk(1, 2)

            def ktile(lo, hi):
                return x0a_sb[0][:, lo:hi]

            B_E = {0: 0, 2: 130, 3: 260}
            B_Q = {u: 390 + k * 128
                   for k, u in enumerate([0, 2, 3, 5, 6, 7])}
            B_R = {0: 1158, 2: 1286, 3: 1414}

            def eslab(b, i):
                if i == 1:
                    return x0a_sb[b][:, 3072:3202]
                if i < 4:
                    o = B_E[i]
                    return x0b_sb[b][:, o:o + 130]
                for g in range(NCH):
                    if i < ECH[g + 1]:
                        return xe_sb[b][g][:, i - ECH[g], :]
                raise AssertionError(i)

            def qslab(b, u):
                if u == 1:
                    return x0a_sb[b][:, 3202:3330]
                if u == 4:
                    return x0a_sb[b][:, 3330:3458]
                if u < 8:
                    o = B_Q[u]
                    return x0b_sb[b][:, o:o + 128]
                for g in range(NCH):
                    if u < QCH[g + 1]:
                        return xq_sb[b][g][:, u - QCH[g], :]
                raise AssertionError(u)

            def rslab(b, i):
                if i == 1:
                    return x0a_sb[b][:, 3458:3586]
                if i < 4:
                    o = B_R[i]
                    return x0b_sb[b][:, o:o + 128]
                for g in range(NCH):
                    if i < ECH[g + 1]:
                        return xr_sb[b][g][:, i - ECH[g], :]
                raise AssertionError(i)

            # Pair processing order: 1..56 in 8-pair groups (16-col DMAs),
            # then 57-60, 61-62, 63, 0 (wrap columns). Batch 1 runs the
            # wrap pairs mid-stream (its late chunks are resident by then)
            # so the kernel tail ends on small DMAs.
            groups = [list(range(1 + 8 * g, 9 + 8 * g)) for g in range(7)]
            groups.append([57, 58, 59, 60])
            groups.append([61])
            groups.append([62])
            groups.append([63])
            groups.append([0])
            b1_groups = groups[0:4] + [groups[10], groups[11]] + groups[4:10]

            for b in range(BPC):
                for gi, grp in enumerate(groups if b == 0 else b1_groups):
                    ob = outp.tile([128, 16, F], bf16, tag="ob")
                    for j, i in enumerate(grp):
                        po = ps.tile([128, 512], f32, tag="po")
                        e_i = eslab(b, i)
                        for d in range(3):
                            off = 2 - d
                            nc.tensor.matmul(po[:], e_i[:, off:off + 128],
                                             ktile(d * 512, (d + 1) * 512),
                                             start=(d == 0), stop=False)
                        nc.tensor.matmul(po[:, 0:256],
                                         qslab(b, (2 * i - 1) % W),
                                         ktile(1536, 1792),
                                         start=False, stop=False)
                        nc.tensor.matmul(po[:, 256:512],
                                         qslab(b, (2 * i + 2) % W),
                                         ktile(2048, 2304),
                                         start=False, stop=False)
                        nc.tensor.matmul(po[:], rslab(b, i), ktile(2560, 3072),
                                         start=False, stop=True)
                        nc.vector.tensor_add(ob[:, 2 * j:2 * j + 2, :],
                                             po[:], biasf[:])
                    w0 = 2 * grp[0]
                    nc.scalar.dma_start(out_d[b, :, w0:w0 + 2 * len(grp), :],
                                        ob[:, 0:2 * len(grp), :])

    nc.compile()
    return nc


_NC_CACHE = {}


def _get_nc():
    if "nc" not in _NC_CACHE:
        _NC_CACHE["nc"] = _build_module()
    return _NC_CACHE["nc"]


def _pack_inputs(x, kern, bias):
    import ml_dtypes

    bf16 = ml_dtypes.bfloat16
    x = np.asarray(x, dtype=np.float32)
    kern = np.asarray(kern, dtype=np.float32)
    bias = np.asarray(bias, dtype=np.float32)

    xt = np.transpose(x, (0, 2, 3, 1))          # [B, W, C, H]
    # xe: [B, p=(wp*64+c), i, h'] with h' = 130 circularly padded h
    xpad = np.concatenate([xt[..., H - 1:H], xt, xt[..., 0:1]], axis=-1)
    xe = (xpad.reshape(B, NP, 2, C, 130)
          .transpose(0, 2, 3, 1, 4)
          .reshape(B, 128, NP, 130)).astype(bf16)
    # xq: [B, p=(s*64+c), u, h], s=0 -> dh=-1 (x[h+1]), s=1 -> dh=0
    xm1 = np.roll(xt, -1, axis=-1)              # x[(h+1)%H]
    xq = (np.stack([xm1, xt], axis=1)           # [B, 2, W, C, H]
          .transpose(0, 1, 3, 2, 4)
          .reshape(B, 128, W, 128)).astype(bf16)
    # xr: [B, p, i, h]: rows 0:64 = col (2i-1)%W @ dh=+1, 64:128 = (2i+2)%W
    xp1 = np.roll(xt, 1, axis=-1)               # x[(h-1)%H]
    i_arr = np.arange(NP)
    top = xp1[:, (2 * i_arr - 1) % W].transpose(0, 2, 1, 3)   # [B, C, NP, H]
    bot = xp1[:, (2 * i_arr + 2) % W].transpose(0, 2, 1, 3)
    xr = np.concatenate([top, bot], axis=1).astype(bf16)      # [B, 128, NP, H]

    # kernel tiles [128, 6, 512]
    kw3 = kern.reshape(9, C, F)
    kt = np.zeros((6, 128, 512), dtype=np.float32)
    for d in range(3):
        kt[d, 0:64, 0:256] = kw3[d + 3]
        kt[d, 64:128, 0:256] = kw3[d]
        kt[d, 0:64, 256:512] = kw3[d + 6]
        kt[d, 64:128, 256:512] = kw3[d + 3]
    kt[3, 0:64, 0:256] = kw3[6]
    kt[3, 64:128, 0:256] = kw3[7]
    kt[4, 0:64, 0:256] = kw3[0]
    kt[4, 64:128, 0:256] = kw3[1]
    kt[5, 0:64, 0:256] = kw3[8]
    kt[5, 64:128, 256:512] = kw3[2]
    kt = np.ascontiguousarray(kt.transpose(1, 0, 2)).astype(bf16)

    biasf = np.ascontiguousarray(
        np.broadcast_to(np.tile(bias, 2)[None, :], (128, 512))).astype(
            np.float32)
    ktb = np.broadcast_to(kt.reshape(1, 128, 3072), (B, 128, 3072))
    x0a = np.concatenate([ktb, xe[:, :, 1, :], xq[:, :, 1, :],
                          xq[:, :, 4, :], xr[:, :, 1, :]], axis=-1)
    x0b = np.concatenate([xe[:, :, 0, :], xe[:, :, 2, :], xe[:, :, 3, :],
                          xq[:, :, 0, :], xq[:, :, 2, :], xq[:, :, 3, :],
                          xq[:, :, 5, :], xq[:, :, 6, :], xq[:, :, 7, :],
                          xr[:, :, 0, :], xr[:, :, 2, :], xr[:, :, 3, :]],
                         axis=-1)
    return xe, xq, xr, x0a, x0b, biasf


def kernel(x, kernel, bias, _trace=False):
    from concourse.bass_utils import run_bass_kernel_spmd

    xe, xq, xr, x0a, x0b, biasf = _pack_inputs(x, kernel, bias)

    nc = _get_nc()
    in_maps = [
        {"xe": np.ascontiguousarray(xe[c * BPC:(c + 1) * BPC]),
         "xq": np.ascontiguousarray(xq[c * BPC:(c + 1) * BPC]),
         "xr": np.ascontiguousarray(xr[c * BPC:(c + 1) * BPC]),
         "x0a": np.ascontiguousarray(x0a[c * BPC:(c + 1) * BPC]),
         "x0b": np.ascontiguousarray(x0b[c * BPC:(c + 1) * BPC]),
         "biasf": biasf}
        for c in range(NCORES)
    ]
    res = run_bass_kernel_spmd(nc, in_maps, core_ids=list(range(NCORES)),
                               trace=_trace)
    out = np.concatenate([np.asarray(res.results[c]["out"])
                          for c in range(NCORES)], axis=0)
    if _trace:
        kernel._last_results = res
    return out.astype(np.float32)



# revision 29
# speedup vs baseline: 1.0093x; 1.0093x over previous
"""Trainium2 Bass kernel for circular 3x3 conv — host-packed bf16 GEMM form.

out[b,h,w,f] = sum_{dh,dw,c} x[b,(h-dh)%H,(w-dw)%W,c] * K[j*C+c, f] + bias[f]
with j = dhi + 3*dwi, dh = dhi-1, dw = dwi-1.

Strategy (cost-model-driven): every matmul costs out_free_size cycles on the
PE regardless of contraction depth, so pack the 9*C=576-deep contraction into
as few, widest matmuls as possible. Host pre-packs x into bf16 layouts so
that each output column PAIR (w=2i, 2i+1) accumulates in one PSUM tile
[128h, 512=(2 cols x 256f)] with 2560 streamed N-columns (the structural
floor for bf16 K<=128):

  MM1-3 (d=dhi): lhsT = xe[i] slab (cols 2i,2i+1 on partitions, padded h
         free) sliced at h-offset 2-d; rhs = T_d [128,512] covering
         (dw=0,-1) for col w and (dw=+1,0) for col w+1 — all 4 quadrants
         of the kernel tile are live.
  MM4:   lhsT = xq[2i-1]  (col w-1 at dh=-1,0 stacked); rhs = [K_6;K_7]
         -> po[:,0:256]   (col w's dw=+1 taps, dhi=0,1)
  MM5:   lhsT = xq[2i+2]  (col w+2);  rhs = [K_0;K_1] -> po[:,256:512]
  MM6:   lhsT = xr[i] = [col 2i-1 @ dh=+1 ; col 2i+2 @ dh=+1];
         rhs = [[K_8,0],[0,K_2]] (zero-masked) -> po[:,0:512]

The end-to-end critical path is: entry + (kernel tiles + first two pairs'
slabs DMA bytes) + dma-sem, then 127 pairs of back-to-back matmuls, then the
last pair's bias + out-DMA chain. So the kernel tiles are deduped across
batches (2048+256 half-K payload, no zero quadrants), and chunk-0 x data is
staged as per-pair 514-column packs so each early pair becomes runnable with
minimal leading bytes. DVE adds bias and casts to bf16; bulk 8-column DMAs
write out. Host casts the bf16 result back to fp32.
"""
import numpy as np

B, H, W, C, F = 16, 128, 128, 64, 256
NCORES = 8
BPC = B // NCORES   # batches per core
NP = W // 2         # column pairs per batch
NPACK = 8           # pairs 1..NPACK fed from per-pair packs


def _build_module():
    import concourse.bacc as bacc
    import concourse.mybir as mybir
    import concourse.tile as tile

    f32 = mybir.dt.float32
    bf16 = mybir.dt.bfloat16

    nc = bacc.Bacc("TRN2", target_bir_lowering=False, debug=False,
                   num_devices=NCORES)
    xe_d = nc.dram_tensor("xe", [BPC, 128, NP, 130], bf16,
                          kind="ExternalInput").ap()
    xq_d = nc.dram_tensor("xq", [BPC, 128, W, 128], bf16,
                          kind="ExternalInput").ap()
    xr_d = nc.dram_tensor("xr", [BPC, 128, NP, 128], bf16,
                          kind="ExternalInput").ap()
    # Kernel tiles, shared by both batches:
    #   ktA: [kt0|kt1|kt2] dense [128,1536] for MM1-3, + batch-0's E1 slab
    #        (pair-1's MM1 input rides the first DMA so it can start at
    #        ktA-sem rather than waiting for a second pack sem)
    #   ktB: [kt3L|kt4L|kt5] [128,1024]; kt5 = [[K_8,0],[0,K_2]] block-diag
    #        (the PE rejects partition-offset operands, so MM6 keeps the
    #        zero-masked full-K form)
    ktA_d = nc.dram_tensor("ktA", [128, 1666], bf16,
                           kind="ExternalInput").ap()
    ktB_d = nc.dram_tensor("ktB", [128, 1024], bf16,
                           kind="ExternalInput").ap()
    # Per-pair packs for pairs 1..8: [E_p 130 | q_{2p-1} 128 | q_{2p+2} 128
    # | R_p 128] = 514; wrap pack: [E_0 | q_0 | q_2 | R_0].
    pp_d = nc.dram_tensor("pp", [BPC, NPACK, 128, 514], bf16,
                          kind="ExternalInput").ap()
    wp_d = nc.dram_tensor("wp", [BPC, 128, 514], bf16,
                          kind="ExternalInput").ap()
    biasf_d = nc.dram_tensor("biasf", [128, 512], f32,
                             kind="ExternalInput").ap()
    out_d = nc.dram_tensor("out", [BPC, H, W, F], bf16,
                           kind="ExternalOutput").ap()

    # Geometric chunk boundaries (slab index space): packs cover E/R 1..8,
    # q odd 1..15, q even 4..18, wrap pack E0/q0/q2/R0; chunks stream the
    # rest. xq chunk space is in column units.
    ECH = [9, 16, 40, 64]
    QCH = [17, 32, 80, 128]
    WARMUP = 160

    with tile.TileContext(nc) as tc:
        with (
            tc.tile_pool(name="persist", bufs=1) as persist,
            tc.tile_pool(name="xdbl", bufs=2) as xdbl,
            tc.tile_pool(name="outp", bufs=4) as outp,
            tc.tile_pool(name="ps", bufs=7, space="PSUM") as ps,
            tc.tile_pool(name="psw", bufs=1, space="PSUM") as psw,
        ):
            NCH = len(ECH) - 1
            xe_sb = [[None] * NCH for _ in range(BPC)]
            xq_sb = [[None] * NCH for _ in range(BPC)]
            xr_sb = [[None] * NCH for _ in range(BPC)]

            def load_chunk(b, g):
                pool = xdbl if g < 1 else persist
                e0, e1 = ECH[g], ECH[g + 1]
                q0, q1 = QCH[g], QCH[g + 1]
                te = pool.tile([128, e1 - e0, 130], bf16, tag=f"xe_{g}")
                nc.sync.dma_start(te[:], xe_d[b, :, e0:e1, :])
                tq = pool.tile([128, q1 - q0, 128], bf16, tag=f"xq_{g}")
                nc.sync.dma_start(tq[:], xq_d[b, :, q0:q1, :])
                tr = pool.tile([128, e1 - e0, 128], bf16, tag=f"xr_{g}")
                nc.sync.dma_start(tr[:], xr_d[b, :, e0:e1, :])
                xe_sb[b][g] = te
                xq_sb[b][g] = tq
                xr_sb[b][g] = tr

            if WARMUP:
                # p-state warmup: tiny dep-free matmuls (source is a
                # memset scratch, result goes to a never-read PSUM bank)
                # start right after the entry barrier and keep the PE
                # continuously busy through the cost model's ramp window,
                # ending just as pair 1's data lands.
                wsrc = persist.tile([128, 64], bf16, tag="wsrc")
                nc.vector.memset(wsrc[:], 0.0)
                warm = psw.tile([32, 32], f32, tag="warm")
                for _ in range(WARMUP):
                    nc.tensor.matmul(warm[:], wsrc[:, 0:32], wsrc[:, 32:64],
                                     start=True, stop=True)

            pp_sb = [[None] * NPACK for _ in range(BPC)]
            wp_sb = [None] * BPC

            pp_off = [[0] * NPACK for _ in range(BPC)]

            def load_pack(b, p):
                if b == 0 and p == 1:
                    # batch-0 pair-1's E slab rides in ktA; only Q/R here
                    t = xdbl.tile([128, 384], bf16, tag="pp1s")
                    nc.sync.dma_start(t[:], pp_d[0, 0, :, 130:514])
                    pp_off[0][0] = 130
                else:
                    t = xdbl.tile([128, 514], bf16, tag=f"pp_{p}")
                    nc.sync.dma_start(t[:], pp_d[b, p - 1, :, :])
                pp_sb[b][p - 1] = t

            # Critical-path head: kernel tiles + pair-1 pack + pair-2 pack
            # first, then the rest of batch-0's packs and chunks.
            ktA = persist.tile([128, 1666], bf16, tag="ktA")
            nc.sync.dma_start(ktA[:], ktA_d[:])
            load_pack(0, 1)
            load_pack(0, 2)
            ktB = persist.tile([128, 1024], bf16, tag="ktB")
            nc.sync.dma_start(ktB[:], ktB_d[:])
            for p in range(3, 5):
                load_pack(0, p)
            biasf = persist.tile([128, 512], f32, tag="biasf")
            nc.sync.dma_start(biasf[:], biasf_d[:])
            for p in range(5, NPACK + 1):
                load_pack(0, p)
            load_chunk(0, 0)
            load_chunk(0, 1)
            load_chunk(0, 2)
            tw = xdbl.tile([128, 514], bf16, tag="wp")
            nc.sync.dma_start(tw[:], wp_d[0, :, :])
            wp_sb[0] = tw
            for p in range(1, NPACK + 1):
                load_pack(1, p)
            tw = xdbl.tile([128, 514], bf16, tag="wp")
            nc.sync.dma_start(tw[:], wp_d[1, :, :])
            wp_sb[1] = tw
            load_chunk(1, 0)
            load_chunk(1, 1)
            load_chunk(1, 2)

            def eslab(b, i):
                if b == 0 and i == 1:
                    return ktA[:, 1536:1666]
                if 1 <= i <= NPACK:
                    return pp_sb[b][i - 1][:, 0:130]
                if i == 0:
                    return wp_sb[b][:, 0:130]
                for g in range(NCH):
                    if i < ECH[g + 1]:
                        return xe_sb[b][g][:, i - ECH[g], :]
                raise AssertionError(i)

            def qslab(b, u):
                if u % 2 == 1 and 1 <= u <= 2 * NPACK - 1:
                    p = (u - 1) // 2
                    o = 130 - pp_off[b][p]
                    return pp_sb[b][p][:, o:o + 128]
                if u % 2 == 0 and 4 <= u <= 2 * NPACK + 2:
                    p = u // 2 - 2
                    o = 258 - pp_off[b][p]
                    return pp_sb[b][p][:, o:o + 128]
                if u == 0:
                    return wp_sb[b][:, 130:258]
                if u == 2:
                    return wp_sb[b][:, 258:386]
                for g in range(NCH):
                    if u < QCH[g + 1]:
                        return xq_sb[b][g][:, u - QCH[g], :]
                raise AssertionError(u)

            def rslab(b, i):
                if 1 <= i <= NPACK:
                    o = 386 - pp_off[b][i - 1]
                    return pp_sb[b][i - 1][:, o:o + 128]
                if i == 0:
                    return wp_sb[b][:, 386:514]
                for g in range(NCH):
                    if i < ECH[g + 1]:
                        return xr_sb[b][g][:, i - ECH[g], :]
                raise AssertionError(i)

            # Pair processing order: 1..56 in 8-pair groups (16-col DMAs),
            # then 57-60, 61-62, 63, 0 (wrap columns). Batch 1 runs the
            # wrap pairs mid-stream (its late chunks are resident by then)
            # so the kernel tail ends on small DMAs.
            groups = [list(range(1 + 8 * g, 9 + 8 * g)) for g in range(7)]
            groups.append([57, 58, 59, 60])
            groups.append([61])
            groups.append([62])
            groups.append([63])
            groups.append([0])
            b1_groups = groups[0:4] + [groups[10], groups[11]] + groups[4:10]

            for b in range(BPC):
                for gi, grp in enumerate(groups if b == 0 else b1_groups):
                    ob = outp.tile([128, 16, F], bf16, tag="ob")
                    for j, i in enumerate(grp):
                        po = ps.tile([128, 512], f32, tag="po")
                        e_i = eslab(b, i)
                        for d in range(3):
                            off = 2 - d
                            nc.tensor.matmul(po[:], e_i[:, off:off + 128],
                                             ktA[:, d * 512:(d + 1) * 512],
                                             start=(d == 0), stop=False)
                        nc.tensor.matmul(po[:, 0:256],
                                         qslab(b, (2 * i - 1) % W),
                                         ktB[:, 0:256],
                                         start=False, stop=False)
                        nc.tensor.matmul(po[:, 256:512],
                                         qslab(b, (2 * i + 2) % W),
                                         ktB[:, 256:512],
                                         start=False, stop=False)
                        nc.tensor.matmul(po[:], rslab(b, i),
                                         ktB[:, 512:1024],
                                         start=False, stop=True)
                        nc.vector.tensor_add(ob[:, 2 * j:2 * j + 2, :],
                                             po[:], biasf[:])
                    w0 = 2 * grp[0]
                    # The very last group's out-DMA chain is the kernel tail;
                    # SP's HWDGE+DGE pipeline is ~140ns shorter than
                    # Activation's, and SP's input DMAs are long done by then.
                    eng = (nc.sync if b == 1 and gi == len(b1_groups) - 1
                           else nc.scalar)
                    eng.dma_start(out_d[b, :, w0:w0 + 2 * len(grp), :],
                                  ob[:, 0:2 * len(grp), :])

    nc.compile()
    return nc


_NC_CACHE = {}


def _get_nc():
    if "nc" not in _NC_CACHE:
        _NC_CACHE["nc"] = _build_module()
    return _NC_CACHE["nc"]


def _pack_inputs(x, kern, bias):
    import ml_dtypes

    bf16 = ml_dtypes.bfloat16
    x = np.asarray(x, dtype=np.float32)
    kern = np.asarray(kern, dtype=np.float32)
    bias = np.asarray(bias, dtype=np.float32)

    xt = np.transpose(x, (0, 2, 3, 1))          # [B, W, C, H]
    # xe: [B, p=(wp*64+c), i, h'] with h' = 130 circularly padded h
    xpad = np.concatenate([xt[..., H - 1:H], xt, xt[..., 0:1]], axis=-1)
    xe = (xpad.reshape(B, NP, 2, C, 130)
          .transpose(0, 2, 3, 1, 4)
          .reshape(B, 128, NP, 130)).astype(bf16)
    # xq: [B, p=(s*64+c), u, h], s=0 -> dh=-1 (x[h+1]), s=1 -> dh=0
    xm1 = np.roll(xt, -1, axis=-1)              # x[(h+1)%H]
    xq = (np.stack([xm1, xt], axis=1)           # [B, 2, W, C, H]
          .transpose(0, 1, 3, 2, 4)
          .reshape(B, 128, W, 128)).astype(bf16)
    # xr: [B, p, i, h]: rows 0:64 = col (2i-1)%W @ dh=+1, 64:128 = (2i+2)%W
    xp1 = np.roll(xt, 1, axis=-1)               # x[(h-1)%H]
    i_arr = np.arange(NP)
    top = xp1[:, (2 * i_arr - 1) % W].transpose(0, 2, 1, 3)   # [B, C, NP, H]
    bot = xp1[:, (2 * i_arr + 2) % W].transpose(0, 2, 1, 3)
    xr = np.concatenate([top, bot], axis=1).astype(bf16)      # [B, 128, NP, H]

    # kernel tiles: ktA = 3 dense [128,512] tiles; ktB = [kt3L|kt4L|kt5]
    kw3 = kern.reshape(9, C, F)
    ktA = np.zeros((128, 3, 512), dtype=np.float32)
    for d in range(3):
        ktA[0:64, d, 0:256] = kw3[d + 3]
        ktA[64:128, d, 0:256] = kw3[d]
        ktA[0:64, d, 256:512] = kw3[d + 6]
        ktA[64:128, d, 256:512] = kw3[d + 3]
    ktA = np.ascontiguousarray(ktA.reshape(128, 1536)).astype(bf16)
    ktB = np.zeros((128, 1024), dtype=np.float32)
    ktB[0:64, 0:256] = kw3[6]
    ktB[64:128, 0:256] = kw3[7]
    ktB[0:64, 256:512] = kw3[0]
    ktB[64:128, 256:512] = kw3[1]
    ktB[0:64, 512:768] = kw3[8]
    ktB[64:128, 768:1024] = kw3[2]
    ktB = np.ascontiguousarray(ktB).astype(bf16)

    # per-pair packs (pairs 1..NPACK) and the wrap pack (pair-0 slabs)
    def pack_for(p):
        return np.concatenate(
            [xe[:, :, p, :], xq[:, :, (2 * p - 1) % W, :],
             xq[:, :, (2 * p + 2) % W, :], xr[:, :, p, :]], axis=-1)

    pp = np.stack([pack_for(p) for p in range(1, NPACK + 1)],
                  axis=1)                        # [B, NPACK, 128, 514]
    wp = np.concatenate([xe[:, :, 0, :], xq[:, :, 0, :], xq[:, :, 2, :],
                         xr[:, :, 0, :]], axis=-1)

    biasf = np.ascontiguousarray(
        np.broadcast_to(np.tile(bias, 2)[None, :], (128, 512))).astype(
            np.float32)
    return xe, xq, xr, ktA, ktB, pp, wp, biasf


def kernel(x, kernel, bias, _trace=False):
    from concourse.bass_utils import run_bass_kernel_spmd

    xe, xq, xr, ktA, ktB, pp, wp, biasf = _pack_inputs(x, kernel, bias)

    nc = _get_nc()
    in_maps = [
        {"xe": np.ascontiguousarray(xe[c * BPC:(c + 1) * BPC]),
         "xq": np.ascontiguousarray(xq[c * BPC:(c + 1) * BPC]),
         "xr": np.ascontiguousarray(xr[c * BPC:(c + 1) * BPC]),
         "pp": np.ascontiguousarray(pp[c * BPC:(c + 1) * BPC]),
         "wp": np.ascontiguousarray(wp[c * BPC:(c + 1) * BPC]),
         "ktA": np.ascontiguousarray(
             np.concatenate([ktA, xe[c * BPC, :, 1, :]], axis=-1)),
         "ktB": ktB, "biasf": biasf}
        for c in range(NCORES)
    ]
    res = run_bass_kernel_spmd(nc, in_maps, core_ids=list(range(NCORES)),
                               trace=_trace)
    out = np.concatenate([np.asarray(res.results[c]["out"])
                          for c in range(NCORES)], axis=0)
    if _trace:
        kernel._last_results = res
    return out.astype(np.float32)
